# revision 6
# baseline (speedup 1.0000x reference)
"""Trainium2 Bass kernel for BasicMambaBlock (B=2, L=1024, DM=1024).

Sharding: tensor-parallel over d_inner (DI=2048 -> 256 channels/core x 8).
Single NEFF; one AllReduce of the x_proj partials ([96, 2048] fp32).
Host does: weight prep/transposes, input replication, final partial-sum
of out_proj partials + residual add (the gather/unshard step).
"""
import os
import sys
import types
import numpy as np
import ml_dtypes

import concourse.bass as bass
import concourse.bacc as bacc
import concourse.tile as tile
from concourse import mybir
from concourse import bass_utils

FP = mybir.dt.float32
BF = mybir.dt.bfloat16
AL = mybir.AluOpType
AF = mybir.ActivationFunctionType

B, L, DM = 2, 1024, 1024
DI = 2 * DM            # 2048
N = 16
K = 4
DTR = DM // 16         # 64
EPS = 1e-5
NCORES = 8
DL = DI // NCORES      # 256 channels per core
NDT = DL // 128        # 2 d-tiles per core
TOK = B * L            # 2048
PAD = 4                # left-pad per sequence in the conv input layout
XIW = 2 * (PAD + L)    # 2056 padded conv-input width

_cache = {}


def _build(a_vec, debug=False):
    nc = bacc.Bacc("TRN2", target_bir_lowering=False, debug=False,
                   num_devices=NCORES)

    # ---------------- dram I/O ----------------
    xT_d = nc.dram_tensor("xT", [DM, TOK], BF, kind="ExternalInput")
    w_in_d = nc.dram_tensor("w_in", [DM, 2 * DL], BF, kind="ExternalInput")
    wsumneg_d = nc.dram_tensor("wsumneg", [2 * NDT, 128], FP, kind="ExternalInput")
    zbias_d = nc.dram_tensor("zbias", [NDT, 128], FP, kind="ExternalInput")
    convdiag_d = nc.dram_tensor("convdiag", [NDT, K, 128, 128], BF, kind="ExternalInput")
    convbias_d = nc.dram_tensor("convbias", [NDT, 128], FP, kind="ExternalInput")
    wxp_d = nc.dram_tensor("wxp", [DL, 96], BF, kind="ExternalInput")
    wdt_d = nc.dram_tensor("wdt", [DTR, DL], BF, kind="ExternalInput")
    dtbias_d = nc.dram_tensor("dtbias", [NDT, 128], FP, kind="ExternalInput")
    ddiag_d = nc.dram_tensor("ddiag", [NDT, 128, 128], BF, kind="ExternalInput")
    ident_d = nc.dram_tensor("ident", [128, 128], BF, kind="ExternalInput")
    wout_d = nc.dram_tensor("wout", [DL, DM], BF, kind="ExternalInput")

    out_d = nc.dram_tensor("out_part", [DM, TOK], FP, kind="ExternalOutput")
    dbg = {}
    if debug:
        dbg["delta"] = nc.dram_tensor("dbg_delta", [DL, TOK], FP, kind="ExternalOutput")
        dbg["u"] = nc.dram_tensor("dbg_u", [DL, TOK], BF, kind="ExternalOutput")
        dbg["sz"] = nc.dram_tensor("dbg_sz", [DL, TOK], BF, kind="ExternalOutput")
        dbg["xdbl"] = nc.dram_tensor("dbg_xdbl", [96, TOK], FP, kind="ExternalOutput")
        dbg["ysz"] = nc.dram_tensor("dbg_ysz", [DL, TOK], BF, kind="ExternalOutput")
        dbg["xz"] = nc.dram_tensor("dbg_xz", [2 * DL, TOK], BF, kind="ExternalOutput")

    with tile.TileContext(nc) as tc:
        from contextlib import ExitStack
        ctx = ExitStack()
        with ctx:
            singles = ctx.enter_context(tc.tile_pool(name="singles", bufs=1))
            dram = ctx.enter_context(tc.tile_pool(name="dram", bufs=1, space="DRAM"))
            bcrows_d = dram.tile([32, TOK], BF)

            # ---- persistent SBUF tiles ----
            xi_pad = [singles.tile([128, XIW], BF, name=f"xi_pad{i}") for i in range(NDT)]
            u_sb = [singles.tile([128, TOK], BF, name=f"u_sb{i}") for i in range(NDT)]
            sz_sb = [singles.tile([128, TOK], BF, name=f"sz_sb{i}") for i in range(NDT)]
            du_sb = [singles.tile([128, TOK], BF, name=f"du_sb{i}") for i in range(NDT)]
            delta_sb = [singles.tile([128, TOK], FP, name=f"delta_sb{i}") for i in range(NDT)]
            ysz_sb = [singles.tile([128, TOK], BF, name=f"ysz_sb{i}") for i in range(NDT)]
            xdbl_sb = singles.tile([96, TOK], FP)
            bc_bf = singles.tile([32, TOK], BF)
            dtrows_bf = singles.tile([DTR, TOK], BF)

            # weights
            w_in_sb = [singles.tile([128, 2 * DL], BF, name=f"w_in_sb{i}") for i in range(DM // 128)]
            for kt in range(DM // 128):
                nc.sync.dma_start(out=w_in_sb[kt][:], in_=w_in_d.ap()[kt * 128:(kt + 1) * 128, :])
            wxp_sb = [singles.tile([128, 96], BF, name=f"wxp_sb{i}") for i in range(NDT)]
            for kt in range(NDT):
                nc.sync.dma_start(out=wxp_sb[kt][:], in_=wxp_d.ap()[kt * 128:(kt + 1) * 128, :])
            wdt_sb = singles.tile([DTR, DL], BF)
            nc.sync.dma_start(out=wdt_sb[:], in_=wdt_d.ap())
            wout_sb = [singles.tile([128, DM], BF, name=f"wout_sb{i}") for i in range(NDT)]
            for kt in range(NDT):
                nc.sync.dma_start(out=wout_sb[kt][:], in_=wout_d.ap()[kt * 128:(kt + 1) * 128, :])
            convdiag_sb = [[singles.tile([128, 128], BF, name=f"cvd{i}_{k}")
                            for k in range(K)] for i in range(NDT)]
            for i in range(NDT):
                for k in range(K):
                    nc.sync.dma_start(out=convdiag_sb[i][k][:], in_=convdiag_d.ap()[i, k, :, :])
            ddiag_sb = [singles.tile([128, 128], BF, name=f"ddiag{i}") for i in range(NDT)]
            for i in range(NDT):
                nc.sync.dma_start(out=ddiag_sb[i][:], in_=ddiag_d.ap()[i, :, :])
            ident_sb = singles.tile([128, 128], BF)
            nc.sync.dma_start(out=ident_sb[:], in_=ident_d.ap())

            wsumneg_sb = singles.tile([128, 2 * NDT], FP)
            for m in range(2 * NDT):
                nc.sync.dma_start(out=wsumneg_sb[:, m:m + 1], in_=wsumneg_d.ap()[m:m + 1, :])
            zbias_sb = singles.tile([128, NDT], FP)
            for i in range(NDT):
                nc.sync.dma_start(out=zbias_sb[:, i:i + 1], in_=zbias_d.ap()[i:i + 1, :])
            convbias_sb = singles.tile([128, NDT], FP)
            for i in range(NDT):
                nc.sync.dma_start(out=convbias_sb[:, i:i + 1], in_=convbias_d.ap()[i:i + 1, :])
            dtbias_sb = singles.tile([128, NDT], FP)
            for i in range(NDT):
                nc.sync.dma_start(out=dtbias_sb[:, i:i + 1], in_=dtbias_d.ap()[i:i + 1, :])

            ones_sb = singles.tile([128, 1], BF)
            nc.vector.memset(ones_sb[:], 1.0)

            def pbcast(row_ap, parts=128):
                return bass.AP(tensor=row_ap.tensor, offset=row_ap.offset,
                               ap=[[0, parts]] + [list(d) for d in row_ap.ap[1:]])

            # ================= S1: load x, LN stats =================
            with tc.tile_pool(name="xtp", bufs=1) as xtp, \
                 tc.tile_pool(name="sqp", bufs=2) as sqp, \
                 tc.tile_pool(name="psA", bufs=4, space="PSUM") as psA:
                rstd_b = xtp.tile([128, TOK], FP)
                musr_b = xtp.tile([128, TOK], FP)
                s_a = xtp.tile([1, TOK], FP)
                s_b = xtp.tile([1, TOK], FP)
                s_c = xtp.tile([1, TOK], FP)
                xT_sb = [xtp.tile([128, TOK], BF, name=f"xT{i}") for i in range(DM // 128)]
                for kt in range(DM // 128):
                    nc.sync.dma_start(out=xT_sb[kt][:], in_=xT_d.ap()[kt * 128:(kt + 1) * 128, :])

                NKT = DM // 128
                inv = 1.0 / DM
                for f in range(4):
                    fs = slice(f * 512, (f + 1) * 512)
                    ps_s = psA.tile([1, 512], FP, name="ps_s", bufs=2)
                    ps_q = psA.tile([1, 512], FP, name="ps_q", bufs=2)
                    for kt in range(NKT):
                        sq = sqp.tile([128, 512], BF, name="sq")
                        nc.scalar.activation(sq[:], xT_sb[kt][:, fs], AF.Square)
                        nc.tensor.matmul(ps_s[:], ones_sb[:], xT_sb[kt][:, fs],
                                         start=(kt == 0), stop=(kt == NKT - 1))
                        nc.tensor.matmul(ps_q[:], ones_sb[:], sq[:],
                                         start=(kt == 0), stop=(kt == NKT - 1))
                    sa, sb_, sc = s_a[:, fs], s_b[:, fs], s_c[:, fs]
                    nc.vector.tensor_scalar(sa, ps_s[:], inv, None, AL.mult)      # mu
                    nc.vector.tensor_scalar(sb_, ps_q[:], inv, EPS, AL.mult, AL.add)
                    nc.vector.tensor_mul(sc, sa, sa)                              # mu^2
                    nc.vector.tensor_tensor(sb_, sb_, sc, AL.subtract)            # var+eps
                    nc.scalar.activation(sc, sb_, AF.Ln)
                    nc.scalar.activation(sb_, sc, AF.Exp, scale=-0.5)             # rstd
                    nc.vector.tensor_mul(sa, sa, sb_)                             # mu*rstd
                stat_bounce = dram.tile([2, TOK], FP)
                nc.sync.dma_start(out=stat_bounce[0:1, :], in_=s_b[0:1, :])
                nc.sync.dma_start(out=stat_bounce[1:2, :], in_=s_a[0:1, :])
                nc.sync.dma_start(out=rstd_b[:], in_=pbcast(stat_bounce[0:1, :]))
                nc.sync.dma_start(out=musr_b[:], in_=pbcast(stat_bounce[1:2, :]))

                # ============= S2: in_proj + LN fixup =============
                # m-tiles 0..NDT-1 -> xi ; NDT..2*NDT-1 -> z
                for i in range(NDT):
                    nc.vector.memset(xi_pad[i][:], 0.0)
                for mt in range(2 * NDT):
                    for f in range(4):
                        fs = slice(f * 512, (f + 1) * 512)
                        mm = psA.tile([128, 512], FP, name="mm", bufs=4)
                        for kt in range(NKT):
                            nc.tensor.matmul(mm[:], w_in_sb[kt][:, mt * 128:(mt + 1) * 128],
                                             xT_sb[kt][:, fs],
                                             start=(kt == 0), stop=(kt == NKT - 1))
                        t1 = sqp.tile([128, 512], FP, name="fix1")
                        nc.vector.tensor_mul(t1[:], mm[:], rstd_b[:, fs])
                        if mt < NDT:
                            # xi rows -> padded layout
                            b_ = f // 2
                            c0 = (f % 2) * 512
                            base = PAD + b_ * (L + PAD)
                            outap = xi_pad[mt][:, base + c0: base + c0 + 512]
                            nc.vector.scalar_tensor_tensor(
                                outap, musr_b[:, fs], wsumneg_sb[:, mt:mt + 1], t1[:],
                                AL.mult, AL.add)
                        else:
                            zt = sqp.tile([128, 512], BF, name="ztmp")
                            nc.vector.scalar_tensor_tensor(
                                zt[:], musr_b[:, fs], wsumneg_sb[:, mt:mt + 1], t1[:],
                                AL.mult, AL.add)
                            i = mt - NDT
                            nc.scalar.activation(sz_sb[i][:, fs], zt[:], AF.Silu,
                                                 bias=zbias_sb[:, i:i + 1])

            # ============= S3: conv + silu -> u =============
            with tc.tile_pool(name="tmp2", bufs=2) as sqp, \
                 tc.tile_pool(name="psB", bufs=4, space="PSUM") as psA:
                xp_part = sqp.tile([96, TOK], FP, name="xp_part", bufs=1)
                for i in range(NDT):
                    for b_ in range(B):
                        for fc in range(L // 512):
                            cv = psA.tile([128, 512], FP, name="cv", bufs=2)
                            base = PAD + b_ * (L + PAD)
                            c0 = fc * 512
                            for k in range(K):
                                rhs = xi_pad[i][:, base + c0 + k - (K - 1):
                                                base + c0 + k - (K - 1) + 512]
                                nc.tensor.matmul(cv[:], convdiag_sb[i][k][:], rhs,
                                                 start=(k == 0), stop=(k == K - 1))
                            nc.scalar.activation(
                                u_sb[i][:, b_ * L + c0: b_ * L + c0 + 512], cv[:],
                                AF.Silu, bias=convbias_sb[:, i:i + 1])

                # ============= S4: x_proj partial + AllReduce =============
                for f in range(4):
                    fs = slice(f * 512, (f + 1) * 512)
                    xp = psA.tile([96, 512], FP, name="xp", bufs=2)
                    for kt in range(NDT):
                        nc.tensor.matmul(xp[:], wxp_sb[kt][:], u_sb[kt][:, fs],
                                         start=(kt == 0), stop=(kt == NDT - 1))
                    nc.scalar.copy(xp_part[:, fs], xp[:])

                cc_in = dram.tile([96, TOK], FP)
                cc_out = dram.tile([96, TOK], FP, addr_space="Shared")
                nc.sync.dma_start(out=cc_in[:], in_=xp_part[:])
                nc.gpsimd.collective_compute(
                    "AllReduce", AL.add,
                    replica_groups=[list(range(NCORES))],
                    ins=[cc_in.opt()], outs=[cc_out.opt()])
                nc.sync.dma_start(out=xdbl_sb[:], in_=cc_out[:])

                # ============= S5: dt_proj + softplus =============
                nc.vector.tensor_copy(dtrows_bf[:], xdbl_sb[0:DTR, :])
                nc.vector.tensor_copy(bc_bf[:], xdbl_sb[DTR:96, :])
                nc.sync.dma_start(out=bcrows_d[:], in_=bc_bf[:])
                for i in range(NDT):
                    for f in range(4):
                        fs = slice(f * 512, (f + 1) * 512)
                        dtp = psA.tile([128, 512], FP, name="dtp", bufs=2)
                        nc.tensor.matmul(dtp[:], wdt_sb[:, i * 128:(i + 1) * 128],
                                         dtrows_bf[:, fs], start=True, stop=True)
                        e1 = sqp.tile([128, 512], FP, name="e1")
                        nc.scalar.activation(e1[:], dtp[:], AF.Exp,
                                             bias=dtbias_sb[:, i:i + 1])
                        nc.scalar.activation(delta_sb[i][:, fs], e1[:], AF.Ln, bias=1.0)

            # du = delta * u
            for i in range(NDT):
                nc.vector.tensor_mul(du_sb[i][:], delta_sb[i][:], u_sb[i][:])

            if debug:
                for i in range(NDT):
                    nc.sync.dma_start(out=dbg["delta"].ap()[i * 128:(i + 1) * 128, :], in_=delta_sb[i][:])
                    nc.sync.dma_start(out=dbg["u"].ap()[i * 128:(i + 1) * 128, :], in_=u_sb[i][:])
                    nc.sync.dma_start(out=dbg["sz"].ap()[i * 128:(i + 1) * 128, :], in_=sz_sb[i][:])
                nc.sync.dma_start(out=dbg["xdbl"].ap(), in_=xdbl_sb[:])

            # ============= S7: scan section =============
            with tc.tile_pool(name="psY", bufs=1, space="PSUM") as psY, \
                 tc.tile_pool(name="bbp", bufs=3) as bbp, \
                 tc.tile_pool(name="ccp", bufs=3) as ccp, \
                 tc.tile_pool(name="dap", bufs=3) as dap, \
                 tc.tile_pool(name="dbup", bufs=3) as dbup, \
                 tc.tile_pool(name="hp", bufs=3) as hp, \
                 tc.tile_pool(name="gp", bufs=3) as gp:
                y_ps = [psY.tile([128, TOK], FP, name=f"y_ps{i}") for i in range(NDT)]
                for n in range(N):
                    Bb = bbp.tile([128, TOK], BF, name="Bb")
                    nc.sync.dma_start(out=Bb[:], in_=pbcast(bcrows_d[n:n + 1, :]))
                    Cb = ccp.tile([128, TOK], BF, name="Cb")
                    nc.sync.dma_start(out=Cb[:], in_=pbcast(bcrows_d[N + n:N + n + 1, :]))
                    for i in range(NDT):
                        dA = dap.tile([128, TOK], FP, name="dA")
                        nc.scalar.activation(dA[:], delta_sb[i][:], AF.Exp,
                                             scale=float(a_vec[n]))
                        dBu = dbup.tile([128, TOK], BF, name="dBu")
                        nc.vector.tensor_mul(dBu[:], du_sb[i][:], Bb[:])
                        h = hp.tile([128, TOK], BF, name="h")
                        for b_ in range(B):
                            bs = slice(b_ * L, (b_ + 1) * L)
                            nc.vector.tensor_tensor_scan(
                                h[:, bs], dA[:, bs], dBu[:, bs], 0.0, AL.mult, AL.add)
                        g = gp.tile([128, TOK], BF, name="g")
                        nc.vector.tensor_mul(g[:], h[:], Cb[:])
                        for f in range(4):
                            fs = slice(f * 512, (f + 1) * 512)
                            nc.tensor.matmul(y_ps[i][:, fs], ident_sb[:], g[:, fs],
                                             start=(n == 0), stop=False)
                # += D*u ; then ysz = y * sz
                for i in range(NDT):
                    for f in range(4):
                        fs = slice(f * 512, (f + 1) * 512)
                        nc.tensor.matmul(y_ps[i][:, fs], ddiag_sb[i][:], u_sb[i][:, fs],
                                         start=False, stop=True)
                    for f in range(4):
                        fs = slice(f * 512, (f + 1) * 512)
                        nc.vector.tensor_mul(ysz_sb[i][:, fs], y_ps[i][:, fs], sz_sb[i][:, fs])

            if debug:
                for i in range(NDT):
                    nc.sync.dma_start(out=dbg["ysz"].ap()[i * 128:(i + 1) * 128, :], in_=ysz_sb[i][:])

            # ============= S9: out_proj partial =============
            with tc.tile_pool(name="psO", bufs=4, space="PSUM") as psO, \
                 tc.tile_pool(name="osp", bufs=4) as osp:
                for m in range(DM // 128):
                    for f in range(4):
                        fs = slice(f * 512, (f + 1) * 512)
                        po = psO.tile([128, 512], FP, name="po")
                        for kt in range(NDT):
                            nc.tensor.matmul(po[:], wout_sb[kt][:, m * 128:(m + 1) * 128],
                                             ysz_sb[kt][:, fs],
                                             start=(kt == 0), stop=(kt == NDT - 1))
                        ost = osp.tile([128, 512], FP, name="ost")
                        nc.scalar.copy(ost[:], po[:])
                        nc.sync.dma_start(out=out_d.ap()[m * 128:(m + 1) * 128, fs], in_=ost[:])

    nc.compile()
    return nc


def _prep_inputs(inputs):
    """Host-side weight prep. Returns per-core input maps."""
    f32 = np.float32
    bf16 = ml_dtypes.bfloat16
    x = np.asarray(inputs["x"], f32)
    ln_g = np.asarray(inputs["ln_g"], f32)
    ln_b = np.asarray(inputs["ln_b"], f32)
    W = np.asarray(inputs["in_proj_w"], f32)
    conv_w = np.asarray(inputs["conv_w"], f32)
    conv_b = np.asarray(inputs["conv_b"], f32)
    xpw = np.asarray(inputs["x_proj_w"], f32)
    dtw = np.asarray(inputs["dt_proj_w"], f32)
    dtb = np.asarray(inputs["dt_proj_b"], f32)
    A_log = np.asarray(inputs["A_log"], f32)
    Dv = np.asarray(inputs["D"], f32)
    ow = np.asarray(inputs["out_proj_w"], f32)

    a_full = -np.exp(A_log)          # (DI, N)
    assert np.allclose(a_full, a_full[0:1, :], rtol=1e-5), \
        "kernel assumes A shared across channels"
    a_vec = a_full[0]                # (N,)

    Wg = W * ln_g[None, :]           # (2*DI, DM)
    bvec = W @ ln_b                  # (2*DI,)

    xT = np.ascontiguousarray(x.transpose(2, 0, 1).reshape(DM, TOK)).astype(bf16)
    ident = np.eye(128, dtype=bf16)

    in_maps = []
    for core in range(NCORES):
        d0 = DL * core
        sl = slice(d0, d0 + DL)
        rows = np.r_[d0:d0 + DL, DI + d0:DI + d0 + DL]
        w_in_T = np.ascontiguousarray(Wg[rows].T).astype(bf16)          # (DM, 2*DL)
        wsumneg = (-Wg[rows].sum(axis=1)).astype(f32).reshape(2 * NDT, 128)
        zbias = bvec[DI + d0:DI + d0 + DL].astype(f32).reshape(NDT, 128)
        xi_bias = bvec[d0:d0 + DL]
        cw = conv_w[sl, 0, :]                                           # (DL, K)
        conv_b2 = (conv_b[sl] + xi_bias * cw.sum(-1)).astype(f32).reshape(NDT, 128)
        convdiag = np.zeros((NDT, K, 128, 128), bf16)
        for i in range(NDT):
            for k in range(K):
                np.fill_diagonal(convdiag[i, k], cw[i * 128:(i + 1) * 128, k].astype(bf16))
        wxp = np.ascontiguousarray(xpw[:, sl].T).astype(bf16)           # (DL, 96)
        wdt = np.ascontiguousarray(dtw[sl, :].T).astype(bf16)           # (DTR, DL)
        dtbias = dtb[sl].astype(f32).reshape(NDT, 128)
        ddiag = np.zeros((NDT, 128, 128), bf16)
        for i in range(NDT):
            np.fill_diagonal(ddiag[i], Dv[sl][i * 128:(i + 1) * 128].astype(bf16))
        wout = np.ascontiguousarray(ow[:, sl].T).astype(bf16)           # (DL, DM)

        in_maps.append({
            "xT": xT, "w_in": w_in_T, "wsumneg": wsumneg, "zbias": zbias,
            "convdiag": convdiag, "convbias": conv_b2, "wxp": wxp,
            "wdt": wdt, "dtbias": dtbias, "ddiag": ddiag, "ident": ident,
            "wout": wout,
        })
    return a_vec, in_maps, x


def run(inputs, trace=False, debug=False):
    a_vec, in_maps, x = _prep_inputs(inputs)
    key = (a_vec.tobytes(), debug)
    if key not in _cache:
        _cache[key] = _build(a_vec, debug=debug)
    nc = _cache[key]
    res = bass_utils.run_bass_kernel_spmd(
        nc, in_maps, core_ids=list(range(NCORES)), trace=trace,
        trace_cores=list(range(NCORES)) if trace else None)
    acc = np.zeros((DM, TOK), np.float32)
    for r in res.results:
        acc += r["out_part"]
    out = x + acc.reshape(DM, B, L).transpose(1, 2, 0)
    return out, res


def kernel(**inputs):
    out, _ = run(inputs, trace=False, debug=False)
    return out


# revision 7
# speedup vs baseline: 1.0101x; 1.0101x over previous
"""Trainium2 Bass kernel for BasicMambaBlock (B=2, L=1024, DM=1024).

Sharding: tensor-parallel over d_inner (DI=2048 -> 256 channels/core x 8).
Single NEFF; one AllReduce of the x_proj partials ([96, 2048] fp32).
Host does: weight prep/transposes, input replication, final partial-sum
of out_proj partials + residual add (the gather/unshard step).
"""
import os
import sys
import types
import numpy as np
import ml_dtypes

import concourse.bass as bass
import concourse.bacc as bacc
import concourse.tile as tile
from concourse import mybir
from concourse import bass_utils

FP = mybir.dt.float32
BF = mybir.dt.bfloat16
AL = mybir.AluOpType
AF = mybir.ActivationFunctionType

B, L, DM = 2, 1024, 1024
DI = 2 * DM            # 2048
N = 16
K = 4
DTR = DM // 16         # 64
EPS = 1e-5
NCORES = 8
DL = DI // NCORES      # 256 channels per core
NDT = DL // 128        # 2 d-tiles per core
TOK = B * L            # 2048
PAD = 4                # left-pad per sequence in the conv input layout
XIW = 2 * (PAD + L)    # 2056 padded conv-input width

_cache = {}


def _build(a_vec, debug=False):
    nc = bacc.Bacc("TRN2", target_bir_lowering=False, debug=False,
                   num_devices=NCORES)

    # ---------------- dram I/O ----------------
    xT_d = nc.dram_tensor("xT", [DM, TOK], BF, kind="ExternalInput")
    w_in_d = nc.dram_tensor("w_in", [DM, 2 * DL], BF, kind="ExternalInput")
    wsumneg_d = nc.dram_tensor("wsumneg", [2 * NDT, 128], FP, kind="ExternalInput")
    zbias_d = nc.dram_tensor("zbias", [NDT, 128], FP, kind="ExternalInput")
    convdiag_d = nc.dram_tensor("convdiag", [NDT, K, 128, 128], BF, kind="ExternalInput")
    convbias_d = nc.dram_tensor("convbias", [NDT, 128], FP, kind="ExternalInput")
    wxp_d = nc.dram_tensor("wxp", [DL, 96], BF, kind="ExternalInput")
    wdt_d = nc.dram_tensor("wdt", [DTR, DL], BF, kind="ExternalInput")
    dtbias_d = nc.dram_tensor("dtbias", [NDT, 128], FP, kind="ExternalInput")
    ddiag_d = nc.dram_tensor("ddiag", [NDT, 128, 128], BF, kind="ExternalInput")
    ident_d = nc.dram_tensor("ident", [128, 128], BF, kind="ExternalInput")
    wout_d = nc.dram_tensor("wout", [DL, DM], BF, kind="ExternalInput")

    out_d = nc.dram_tensor("out_part", [DM, TOK], FP, kind="ExternalOutput")
    dbg = {}
    if debug:
        dbg["delta"] = nc.dram_tensor("dbg_delta", [DL, TOK], FP, kind="ExternalOutput")
        dbg["u"] = nc.dram_tensor("dbg_u", [DL, TOK], BF, kind="ExternalOutput")
        dbg["sz"] = nc.dram_tensor("dbg_sz", [DL, TOK], BF, kind="ExternalOutput")
        dbg["xdbl"] = nc.dram_tensor("dbg_xdbl", [96, TOK], FP, kind="ExternalOutput")
        dbg["ysz"] = nc.dram_tensor("dbg_ysz", [DL, TOK], BF, kind="ExternalOutput")
        dbg["xz"] = nc.dram_tensor("dbg_xz", [2 * DL, TOK], BF, kind="ExternalOutput")

    with tile.TileContext(nc) as tc:
        from contextlib import ExitStack
        ctx = ExitStack()
        with ctx:
            singles = ctx.enter_context(tc.tile_pool(name="singles", bufs=1))
            dram = ctx.enter_context(tc.tile_pool(name="dram", bufs=1, space="DRAM"))
            bcrows_d = dram.tile([32, TOK], BF)

            # ---- persistent SBUF tiles ----
            xi_pad = [singles.tile([128, XIW], BF, name=f"xi_pad{i}") for i in range(NDT)]
            u_sb = [singles.tile([128, TOK], BF, name=f"u_sb{i}") for i in range(NDT)]
            sz_sb = [singles.tile([128, TOK], BF, name=f"sz_sb{i}") for i in range(NDT)]
            du_sb = [singles.tile([128, TOK], BF, name=f"du_sb{i}") for i in range(NDT)]
            delta_sb = [singles.tile([128, TOK], FP, name=f"delta_sb{i}") for i in range(NDT)]
            ysz_sb = [singles.tile([128, TOK], BF, name=f"ysz_sb{i}") for i in range(NDT)]
            xdbl_sb = singles.tile([96, TOK], FP)
            bc_bf = singles.tile([32, TOK], BF)
            dtrows_bf = singles.tile([DTR, TOK], BF)

            # weights
            w_in_sb = [singles.tile([128, 2 * DL], BF, name=f"w_in_sb{i}") for i in range(DM // 128)]
            for kt in range(DM // 128):
                nc.sync.dma_start(out=w_in_sb[kt][:], in_=w_in_d.ap()[kt * 128:(kt + 1) * 128, :])
            wxp_sb = [singles.tile([128, 96], BF, name=f"wxp_sb{i}") for i in range(NDT)]
            for kt in range(NDT):
                nc.sync.dma_start(out=wxp_sb[kt][:], in_=wxp_d.ap()[kt * 128:(kt + 1) * 128, :])
            wdt_sb = singles.tile([DTR, DL], BF)
            nc.sync.dma_start(out=wdt_sb[:], in_=wdt_d.ap())
            wout_sb = [singles.tile([128, DM], BF, name=f"wout_sb{i}") for i in range(NDT)]
            for kt in range(NDT):
                nc.sync.dma_start(out=wout_sb[kt][:], in_=wout_d.ap()[kt * 128:(kt + 1) * 128, :])
            convdiag_sb = [[singles.tile([128, 128], BF, name=f"cvd{i}_{k}")
                            for k in range(K)] for i in range(NDT)]
            for i in range(NDT):
                for k in range(K):
                    nc.sync.dma_start(out=convdiag_sb[i][k][:], in_=convdiag_d.ap()[i, k, :, :])
            ddiag_sb = [singles.tile([128, 128], BF, name=f"ddiag{i}") for i in range(NDT)]
            for i in range(NDT):
                nc.sync.dma_start(out=ddiag_sb[i][:], in_=ddiag_d.ap()[i, :, :])
            ident_sb = singles.tile([128, 128], BF)
            nc.sync.dma_start(out=ident_sb[:], in_=ident_d.ap())

            wsumneg_sb = singles.tile([128, 2 * NDT], FP)
            for m in range(2 * NDT):
                nc.sync.dma_start(out=wsumneg_sb[:, m:m + 1], in_=wsumneg_d.ap()[m:m + 1, :])
            zbias_sb = singles.tile([128, NDT], FP)
            for i in range(NDT):
                nc.sync.dma_start(out=zbias_sb[:, i:i + 1], in_=zbias_d.ap()[i:i + 1, :])
            convbias_sb = singles.tile([128, NDT], FP)
            for i in range(NDT):
                nc.sync.dma_start(out=convbias_sb[:, i:i + 1], in_=convbias_d.ap()[i:i + 1, :])
            dtbias_sb = singles.tile([128, NDT], FP)
            for i in range(NDT):
                nc.sync.dma_start(out=dtbias_sb[:, i:i + 1], in_=dtbias_d.ap()[i:i + 1, :])

            ones_sb = singles.tile([128, 1], BF)
            nc.vector.memset(ones_sb[:], 1.0)

            def pbcast(row_ap, parts=128):
                return bass.AP(tensor=row_ap.tensor, offset=row_ap.offset,
                               ap=[[0, parts]] + [list(d) for d in row_ap.ap[1:]])

            # ================= S1: load x, LN stats =================
            ctx.enter_context(nc.named_scope("whole"))
            with tc.tile_pool(name="xtp", bufs=1) as xtp, \
                 tc.tile_pool(name="sqp", bufs=2) as sqp, \
                 tc.tile_pool(name="psA", bufs=4, space="PSUM") as psA:
                rstd_b = xtp.tile([128, TOK], FP)
                musr_b = xtp.tile([128, TOK], FP)
                s_a = xtp.tile([1, TOK], FP)
                s_b = xtp.tile([1, TOK], FP)
                s_c = xtp.tile([1, TOK], FP)
                xT_sb = [xtp.tile([128, TOK], BF, name=f"xT{i}") for i in range(DM // 128)]
                for kt in range(DM // 128):
                    nc.sync.dma_start(out=xT_sb[kt][:], in_=xT_d.ap()[kt * 128:(kt + 1) * 128, :])

                NKT = DM // 128
                inv = 1.0 / DM
                for f in range(4):
                    fs = slice(f * 512, (f + 1) * 512)
                    ps_s = psA.tile([1, 512], FP, name="ps_s", bufs=2)
                    ps_q = psA.tile([1, 512], FP, name="ps_q", bufs=2)
                    for kt in range(NKT):
                        sq = sqp.tile([128, 512], BF, name="sq")
                        nc.scalar.activation(sq[:], xT_sb[kt][:, fs], AF.Square)
                        nc.tensor.matmul(ps_s[:], ones_sb[:], xT_sb[kt][:, fs],
                                         start=(kt == 0), stop=(kt == NKT - 1))
                        nc.tensor.matmul(ps_q[:], ones_sb[:], sq[:],
                                         start=(kt == 0), stop=(kt == NKT - 1))
                    sa, sb_, sc = s_a[:, fs], s_b[:, fs], s_c[:, fs]
                    nc.vector.tensor_scalar(sa, ps_s[:], inv, None, AL.mult)      # mu
                    nc.vector.tensor_scalar(sb_, ps_q[:], inv, EPS, AL.mult, AL.add)
                    nc.vector.tensor_mul(sc, sa, sa)                              # mu^2
                    nc.vector.tensor_tensor(sb_, sb_, sc, AL.subtract)            # var+eps
                    nc.scalar.activation(sc, sb_, AF.Ln)
                    nc.scalar.activation(sb_, sc, AF.Exp, scale=-0.5)             # rstd
                    nc.vector.tensor_mul(sa, sa, sb_)                             # mu*rstd
                stat_bounce = dram.tile([2, TOK], FP)
                nc.sync.dma_start(out=stat_bounce[0:1, :], in_=s_b[0:1, :])
                nc.sync.dma_start(out=stat_bounce[1:2, :], in_=s_a[0:1, :])
                nc.sync.dma_start(out=rstd_b[:], in_=pbcast(stat_bounce[0:1, :]))
                nc.sync.dma_start(out=musr_b[:], in_=pbcast(stat_bounce[1:2, :]))

                # ============= S2: in_proj + LN fixup =============
                # (scope: part of s12)
                # m-tiles 0..NDT-1 -> xi ; NDT..2*NDT-1 -> z
                for i in range(NDT):
                    nc.vector.memset(xi_pad[i][:], 0.0)
                for mt in range(2 * NDT):
                    for f in range(4):
                        fs = slice(f * 512, (f + 1) * 512)
                        mm = psA.tile([128, 512], FP, name="mm", bufs=4)
                        for kt in range(NKT):
                            nc.tensor.matmul(mm[:], w_in_sb[kt][:, mt * 128:(mt + 1) * 128],
                                             xT_sb[kt][:, fs],
                                             start=(kt == 0), stop=(kt == NKT - 1))
                        t1 = sqp.tile([128, 512], FP, name="fix1")
                        nc.vector.tensor_mul(t1[:], mm[:], rstd_b[:, fs])
                        if mt < NDT:
                            # xi rows -> padded layout
                            b_ = f // 2
                            c0 = (f % 2) * 512
                            base = PAD + b_ * (L + PAD)
                            outap = xi_pad[mt][:, base + c0: base + c0 + 512]
                            nc.vector.scalar_tensor_tensor(
                                outap, musr_b[:, fs], wsumneg_sb[:, mt:mt + 1], t1[:],
                                AL.mult, AL.add)
                        else:
                            zt = sqp.tile([128, 512], BF, name="ztmp")
                            nc.vector.scalar_tensor_tensor(
                                zt[:], musr_b[:, fs], wsumneg_sb[:, mt:mt + 1], t1[:],
                                AL.mult, AL.add)
                            i = mt - NDT
                            nc.scalar.activation(sz_sb[i][:, fs], zt[:], AF.Silu,
                                                 bias=zbias_sb[:, i:i + 1])

            # ============= S3: conv + silu -> u =============
            with tc.tile_pool(name="tmp2", bufs=2) as sqp, \
                 tc.tile_pool(name="psB", bufs=4, space="PSUM") as psA:
                xp_part = sqp.tile([96, TOK], FP, name="xp_part", bufs=1)
                for i in range(NDT):
                    for b_ in range(B):
                        for fc in range(L // 512):
                            cv = psA.tile([128, 512], FP, name="cv", bufs=2)
                            base = PAD + b_ * (L + PAD)
                            c0 = fc * 512
                            for k in range(K):
                                rhs = xi_pad[i][:, base + c0 + k - (K - 1):
                                                base + c0 + k - (K - 1) + 512]
                                nc.tensor.matmul(cv[:], convdiag_sb[i][k][:], rhs,
                                                 start=(k == 0), stop=(k == K - 1))
                            nc.scalar.activation(
                                u_sb[i][:, b_ * L + c0: b_ * L + c0 + 512], cv[:],
                                AF.Silu, bias=convbias_sb[:, i:i + 1])

                # ============= S4: x_proj partial + AllReduce =============
                for f in range(4):
                    fs = slice(f * 512, (f + 1) * 512)
                    xp = psA.tile([96, 512], FP, name="xp", bufs=2)
                    for kt in range(NDT):
                        nc.tensor.matmul(xp[:], wxp_sb[kt][:], u_sb[kt][:, fs],
                                         start=(kt == 0), stop=(kt == NDT - 1))
                    nc.scalar.copy(xp_part[:, fs], xp[:])

                cc_in = dram.tile([96, TOK], FP)
                cc_out = dram.tile([96, TOK], FP, addr_space="Shared")
                nc.sync.dma_start(out=cc_in[:], in_=xp_part[:])
                nc.gpsimd.collective_compute(
                    "AllReduce", AL.add,
                    replica_groups=[list(range(NCORES))],
                    ins=[cc_in.opt()], outs=[cc_out.opt()])
                nc.sync.dma_start(out=xdbl_sb[:], in_=cc_out[:])

                # ============= S5: dt_proj + softplus =============
                nc.vector.tensor_copy(dtrows_bf[:], xdbl_sb[0:DTR, :])
                nc.vector.tensor_copy(bc_bf[:], xdbl_sb[DTR:96, :])
                nc.sync.dma_start(out=bcrows_d[:], in_=bc_bf[:])
                for i in range(NDT):
                    for f in range(4):
                        fs = slice(f * 512, (f + 1) * 512)
                        dtp = psA.tile([128, 512], FP, name="dtp", bufs=2)
                        nc.tensor.matmul(dtp[:], wdt_sb[:, i * 128:(i + 1) * 128],
                                         dtrows_bf[:, fs], start=True, stop=True)
                        e1 = sqp.tile([128, 512], FP, name="e1")
                        nc.scalar.activation(e1[:], dtp[:], AF.Exp,
                                             bias=dtbias_sb[:, i:i + 1])
                        nc.scalar.activation(delta_sb[i][:, fs], e1[:], AF.Ln, bias=1.0)

            # du = delta * u
            for i in range(NDT):
                nc.vector.tensor_mul(du_sb[i][:], delta_sb[i][:], u_sb[i][:])

            if debug:
                for i in range(NDT):
                    nc.sync.dma_start(out=dbg["delta"].ap()[i * 128:(i + 1) * 128, :], in_=delta_sb[i][:])
                    nc.sync.dma_start(out=dbg["u"].ap()[i * 128:(i + 1) * 128, :], in_=u_sb[i][:])
                    nc.sync.dma_start(out=dbg["sz"].ap()[i * 128:(i + 1) * 128, :], in_=sz_sb[i][:])
                nc.sync.dma_start(out=dbg["xdbl"].ap(), in_=xdbl_sb[:])

            # ============= S7: scan section =============
            with tc.tile_pool(name="psY", bufs=1, space="PSUM") as psY, \
                 tc.tile_pool(name="bbp", bufs=3) as bbp, \
                 tc.tile_pool(name="ccp", bufs=3) as ccp, \
                 tc.tile_pool(name="dap", bufs=3) as dap, \
                 tc.tile_pool(name="dbup", bufs=3) as dbup, \
                 tc.tile_pool(name="hp", bufs=3) as hp, \
                 tc.tile_pool(name="gp", bufs=3) as gp:
                y_ps = [psY.tile([128, TOK], FP, name=f"y_ps{i}") for i in range(NDT)]
                scan_scope = nc.named_scope("scan_section")
                scan_scope.__enter__()
                for n in range(N):
                    Bb = bbp.tile([128, TOK], BF, name="Bb")
                    nc.sync.dma_start(out=Bb[:], in_=pbcast(bcrows_d[n:n + 1, :]))
                    Cb = ccp.tile([128, TOK], BF, name="Cb")
                    nc.sync.dma_start(out=Cb[:], in_=pbcast(bcrows_d[N + n:N + n + 1, :]))
                    for i in range(NDT):
                        dA = dap.tile([128, TOK], FP, name="dA")
                        nc.scalar.activation(dA[:], delta_sb[i][:], AF.Exp,
                                             scale=float(a_vec[n]))
                        dBu = dbup.tile([128, TOK], BF, name="dBu")
                        nc.vector.tensor_mul(dBu[:], du_sb[i][:], Bb[:])
                        h = hp.tile([128, TOK], BF, name="h")
                        for b_ in range(B):
                            bs = slice(b_ * L, (b_ + 1) * L)
                            nc.vector.tensor_tensor_scan(
                                h[:, bs], dA[:, bs], dBu[:, bs], 0.0, AL.mult, AL.add)
                        g = gp.tile([128, TOK], BF, name="g")
                        nc.vector.tensor_mul(g[:], h[:], Cb[:])
                        for f in range(4):
                            fs = slice(f * 512, (f + 1) * 512)
                            nc.tensor.matmul(y_ps[i][:, fs], ident_sb[:], g[:, fs],
                                             start=(n == 0), stop=False)
                scan_scope.__exit__(None, None, None)
                # += D*u ; then ysz = y * sz
                for i in range(NDT):
                    for f in range(4):
                        fs = slice(f * 512, (f + 1) * 512)
                        nc.tensor.matmul(y_ps[i][:, fs], ddiag_sb[i][:], u_sb[i][:, fs],
                                         start=False, stop=True)
                    for f in range(4):
                        fs = slice(f * 512, (f + 1) * 512)
                        nc.vector.tensor_mul(ysz_sb[i][:, fs], y_ps[i][:, fs], sz_sb[i][:, fs])

            if debug:
                for i in range(NDT):
                    nc.sync.dma_start(out=dbg["ysz"].ap()[i * 128:(i + 1) * 128, :], in_=ysz_sb[i][:])

            # ============= S9: out_proj partial =============
            with tc.tile_pool(name="psO", bufs=4, space="PSUM") as psO, \
                 tc.tile_pool(name="osp", bufs=4) as osp:
                for m in range(DM // 128):
                    for f in range(4):
                        fs = slice(f * 512, (f + 1) * 512)
                        po = psO.tile([128, 512], FP, name="po")
                        for kt in range(NDT):
                            nc.tensor.matmul(po[:], wout_sb[kt][:, m * 128:(m + 1) * 128],
                                             ysz_sb[kt][:, fs],
                                             start=(kt == 0), stop=(kt == NDT - 1))
                        ost = osp.tile([128, 512], FP, name="ost")
                        nc.scalar.copy(ost[:], po[:])
                        nc.sync.dma_start(out=out_d.ap()[m * 128:(m + 1) * 128, fs], in_=ost[:])

    nc.compile()
    return nc


def _prep_inputs(inputs):
    """Host-side weight prep. Returns per-core input maps."""
    f32 = np.float32
    bf16 = ml_dtypes.bfloat16
    x = np.asarray(inputs["x"], f32)
    ln_g = np.asarray(inputs["ln_g"], f32)
    ln_b = np.asarray(inputs["ln_b"], f32)
    W = np.asarray(inputs["in_proj_w"], f32)
    conv_w = np.asarray(inputs["conv_w"], f32)
    conv_b = np.asarray(inputs["conv_b"], f32)
    xpw = np.asarray(inputs["x_proj_w"], f32)
    dtw = np.asarray(inputs["dt_proj_w"], f32)
    dtb = np.asarray(inputs["dt_proj_b"], f32)
    A_log = np.asarray(inputs["A_log"], f32)
    Dv = np.asarray(inputs["D"], f32)
    ow = np.asarray(inputs["out_proj_w"], f32)

    a_full = -np.exp(A_log)          # (DI, N)
    assert np.allclose(a_full, a_full[0:1, :], rtol=1e-5), \
        "kernel assumes A shared across channels"
    a_vec = a_full[0]                # (N,)

    Wg = W * ln_g[None, :]           # (2*DI, DM)
    bvec = W @ ln_b                  # (2*DI,)

    xT = np.ascontiguousarray(x.transpose(2, 0, 1).reshape(DM, TOK)).astype(bf16)
    ident = np.eye(128, dtype=bf16)

    in_maps = []
    for core in range(NCORES):
        d0 = DL * core
        sl = slice(d0, d0 + DL)
        rows = np.r_[d0:d0 + DL, DI + d0:DI + d0 + DL]
        w_in_T = np.ascontiguousarray(Wg[rows].T).astype(bf16)          # (DM, 2*DL)
        wsumneg = (-Wg[rows].sum(axis=1)).astype(f32).reshape(2 * NDT, 128)
        zbias = bvec[DI + d0:DI + d0 + DL].astype(f32).reshape(NDT, 128)
        xi_bias = bvec[d0:d0 + DL]
        cw = conv_w[sl, 0, :]                                           # (DL, K)
        conv_b2 = (conv_b[sl] + xi_bias * cw.sum(-1)).astype(f32).reshape(NDT, 128)
        convdiag = np.zeros((NDT, K, 128, 128), bf16)
        for i in range(NDT):
            for k in range(K):
                np.fill_diagonal(convdiag[i, k], cw[i * 128:(i + 1) * 128, k].astype(bf16))
        wxp = np.ascontiguousarray(xpw[:, sl].T).astype(bf16)           # (DL, 96)
        wdt = np.ascontiguousarray(dtw[sl, :].T).astype(bf16)           # (DTR, DL)
        dtbias = dtb[sl].astype(f32).reshape(NDT, 128)
        ddiag = np.zeros((NDT, 128, 128), bf16)
        for i in range(NDT):
            np.fill_diagonal(ddiag[i], Dv[sl][i * 128:(i + 1) * 128].astype(bf16))
        wout = np.ascontiguousarray(ow[:, sl].T).astype(bf16)           # (DL, DM)

        in_maps.append({
            "xT": xT, "w_in": w_in_T, "wsumneg": wsumneg, "zbias": zbias,
            "convdiag": convdiag, "convbias": conv_b2, "wxp": wxp,
            "wdt": wdt, "dtbias": dtbias, "ddiag": ddiag, "ident": ident,
            "wout": wout,
        })
    return a_vec, in_maps, x


def run(inputs, trace=False, debug=False):
    a_vec, in_maps, x = _prep_inputs(inputs)
    key = (a_vec.tobytes(), debug)
    if key not in _cache:
        _cache[key] = _build(a_vec, debug=debug)
    nc = _cache[key]
    res = bass_utils.run_bass_kernel_spmd(
        nc, in_maps, core_ids=list(range(NCORES)), trace=trace,
        trace_cores=list(range(NCORES)) if trace else None)
    acc = np.zeros((DM, TOK), np.float32)
    for r in res.results:
        acc += r["out_part"]
    out = x + acc.reshape(DM, B, L).transpose(1, 2, 0)
    return out, res


def kernel(**inputs):
    out, _ = run(inputs, trace=False, debug=False)
    return out
